# revision 11
# baseline (speedup 1.0000x reference)
"""Trainium2 Bass kernel for a GRU CharRNN.

Model: emb = embed[x]; gi = emb @ W_ih.T + b_ih; GRU over S steps with
W_hh/b_hh; logits = hs @ W_fc.T + b_fc.  Shapes: B=64, S=2048, E=128,
H=256, V=256.

Strategy (8 NeuronCores, data-parallel over batch, 8 rows/core):
  * Fold the embedding into the input-gate table on the host:
    G = embed @ W_ih.T + b_ih (+ b_hh for the r/z parts). Since V=256 the
    whole per-step input contribution is a row-lookup in G[256, 768].
  * On device, G rows are fetched per 32-step chunk with dma_gather
    (transpose mode) which lands them directly in the transposed gate
    layout [gate-dim on partitions, (block, step*batch) on free dim].
  * The recurrence runs in "T layout": gh.T[g, b] = W_hh[g, :] @ h[b, :]
    via 12 matmuls of [128x128] x [128x4] per group-step; the gate math
    uses per-partition biases + scalar_tensor_tensor fusion so each step
    costs 2 ACT + 5 DVE instructions per group.
  * The 8 batch rows per core are split into 2 independent groups of 4 so
    the tile scheduler can hide the serial gate-chain latency of one group
    under the matmuls of the other.
  * h is stored in fp16 (weights fp16, PSUM accumulation fp32); hidden
    states stream to DRAM and a final phase computes logits with W_fc
    stationary-free matmuls and writes the fp32 output.
"""

import functools
import sys

import numpy as np

for _p in ("/opt/trn_rl_repo", "/opt/pypackages"):
    if _p not in sys.path:
        sys.path.append(_p)

import concourse.bass as bass  # noqa: E402
import concourse.tile as tile  # noqa: E402
from concourse import bacc, mybir  # noqa: E402

VOCAB, EMBED, HIDDEN = 256, 128, 256
BATCH, SEQ = 64, 2048
NCORES = 8
BCORE = BATCH // NCORES  # 8 batch rows per core
NG = 2  # independent pipeline groups per core
BG = BCORE // NG  # 4 batch rows per group
CHUNK = 32  # recurrence steps per gather / hs-store chunk
H2 = HIDDEN // 128  # 2 hidden-dim blocks
GB = 3 * HIDDEN // 128  # 6 gate blocks (r0 r1 z0 z1 n0 n1)
G3 = 3 * HIDDEN  # 768

LAST_RESULTS = None  # BassKernelResults from the most recent kernel() call

F32 = mybir.dt.float32
F16 = mybir.dt.float16
I16 = mybir.dt.int16
AF = mybir.ActivationFunctionType
ALU = mybir.AluOpType


def _emit_recurrence(
    nc, tc, S, gtab_d, xw_d, whh_d, bhhn_d, wfc_d, bfc_d, logits_d, hlast_d
):
    nch = S // CHUNK
    with tc.tile_pool(name="const", bufs=1) as constp:
        whh_sb = constp.tile([128, H2 * GB * 128], F16)
        nc.sync.dma_start(whh_sb[:], whh_d[:])
        xw_sb = constp.tile([128, S * BCORE // 16], I16)
        nc.sync.dma_start(xw_sb[:], xw_d[:])
        bhhn_sb = constp.tile([128, H2], F32)
        nc.sync.dma_start(bhhn_sb[:], bhhn_d[:])
        h0 = constp.tile([128, H2, BG], F16)
        nc.vector.memset(h0[:], 0.0)
        wfc_sb = constp.tile([128, H2 * VOCAB], F16)
        nc.sync.dma_start(wfc_sb[:], wfc_d[:])
        bfc_sb = constp.tile([1, VOCAB], F16)
        nc.sync.dma_start(bfc_sb[:], bfc_d[:])
        ones_sb = constp.tile([1, 128], F16)
        nc.vector.memset(ones_sb[:], 1.0)

        with (
            tc.tile_pool(name="gi", bufs=3) as gip,
            tc.tile_pool(name="hr", bufs=2) as hrp,
            tc.tile_pool(name="ps", bufs=1, space="PSUM") as psp,
            tc.tile_pool(name="lps", bufs=2, space="PSUM") as lpsp,
            tc.tile_pool(name="gt", bufs=2) as gtp,
            tc.tile_pool(name="lo", bufs=2) as lop,
        ):
            gi_tiles = {}
            hr_tiles = {}

            def issue_gather(g, c):
                t_ = gip.tile([128, GB, CHUNK * BG], F16, tag=f"gi{g}")
                off = (g * nch + c) * (CHUNK * BG // 16)
                nc.gpsimd.dma_gather(
                    t_[:],
                    gtab_d[:],
                    xw_sb[:, off : off + CHUNK * BG // 16],
                    CHUNK * BG,
                    CHUNK * BG,
                    G3,
                    transpose=True,
                )
                gi_tiles[(g, c)] = t_

            for g in range(NG):
                issue_gather(g, 0)
                if nch > 1:
                    issue_gather(g, 1)

            for t in range(S):
                c, tl = divmod(t, CHUNK)
                for g in range(NG):
                    if tl == 0:
                        hr_tiles[(g, c)] = hrp.tile(
                            [128, H2, CHUNK, BG], F16, tag=f"hr{g}", name=f"hr{g}"
                        )
                        if c + 2 < nch:
                            issue_gather(g, c + 2)
                    hr = hr_tiles[(g, c)]
                    gi = gi_tiles[(g, c)]
                    if t == 0:
                        h_prev = h0[:]
                    elif tl == 0:
                        h_prev = hr_tiles[(g, c - 1)][:, :, CHUNK - 1, :]
                    else:
                        h_prev = hr[:, :, tl - 1, :]

                    prz = psp.tile([128, 4, BG], F32, tag=f"prz{g}")
                    pn = psp.tile([128, H2, BG], F32, tag=f"pn{g}")
                    for gb in range(GB):
                        out_ap = prz[:, gb, :] if gb < 4 else pn[:, gb - 4, :]
                        for eta in range(H2):
                            w0 = (eta * GB + gb) * 128
                            nc.tensor.matmul(
                                out_ap,
                                whh_sb[:, w0 : w0 + 128],
                                h_prev[:, eta, :],
                                start=(eta == 0),
                                stop=(eta == H2 - 1),
                            )

                    gslc = slice(tl * BG, (tl + 1) * BG)
                    s_rz = gtp.tile([128, 4, BG], F32, tag=f"srz{g}")
                    nc.vector.tensor_tensor(
                        s_rz[:], prz[:], gi[:, 0:4, gslc], ALU.add
                    )
                    rz = gtp.tile([128, 4, BG], F32, tag=f"rz{g}")
                    nc.scalar.activation(rz[:], s_rz[:], AF.Sigmoid)
                    tn = gtp.tile([128, H2, BG], F32, tag=f"tn{g}")
                    for eta in range(H2):
                        nc.vector.scalar_tensor_tensor(
                            tn[:, eta, :],
                            pn[:, eta, :],
                            bhhn_sb[:, eta : eta + 1],
                            rz[:, eta, :],
                            ALU.add,
                            ALU.mult,
                        )
                    t3 = gtp.tile([128, H2, BG], F32, tag=f"t3{g}")
                    nc.vector.tensor_tensor(
                        t3[:], tn[:], gi[:, 4:6, gslc], ALU.add
                    )
                    nn = gtp.tile([128, H2, BG], F32, tag=f"nn{g}")
                    nc.scalar.activation(nn[:], t3[:], AF.Tanh)
                    dd = gtp.tile([128, H2, BG], F32, tag=f"dd{g}")
                    nc.vector.tensor_tensor(dd[:], h_prev, nn[:], ALU.subtract)
                    ee = gtp.tile([128, H2, BG], F32, tag=f"ee{g}")
                    nc.vector.tensor_tensor(ee[:], rz[:, 2:4, :], dd[:], ALU.mult)
                    nc.vector.tensor_tensor(
                        hr[:, :, tl, :], nn[:], ee[:], ALU.add
                    )

                    if tl == CHUNK - 1:
                        # logits for this chunk straight from the SBUF ring:
                        # out[pair=(tl,b), v] = hs.T-tile @ W_fc.T (+ b_fc via
                        # a K=1 ones-row matmul into the same PSUM bank)
                        psl = lpsp.tile([128, VOCAB], F32, tag=f"lps{g}")
                        for eta in range(H2):
                            nc.tensor.matmul(
                                psl[:],
                                hr[:, eta, :, :],
                                wfc_sb[:, eta * VOCAB : (eta + 1) * VOCAB],
                                start=(eta == 0),
                                stop=False,
                            )
                        nc.tensor.matmul(
                            psl[:],
                            ones_sb[:],
                            bfc_sb[:],
                            start=False,
                            stop=True,
                        )
                        ot = lop.tile([128, VOCAB], F32, tag=f"ot{g}")
                        nc.scalar.copy(ot[:], psl[:])
                        nc.sync.dma_start(
                            logits_d[
                                c * CHUNK : (c + 1) * CHUNK,
                                g * BG : (g + 1) * BG,
                                :,
                            ],
                            ot[:],
                        )

            for g in range(NG):
                hlf = gtp.tile([128, H2, BG], F32, tag=f"hl{g}")
                nc.scalar.copy(
                    hlf[:], hr_tiles[(g, nch - 1)][:, :, CHUNK - 1, :]
                )
                nc.sync.dma_start(
                    hlast_d[:, g * BCORE : (g + 1) * BCORE], hlf[:]
                )


@functools.lru_cache(maxsize=2)
def _build(S):
    nc = bacc.Bacc("TRN2", target_bir_lowering=False, debug=False)
    gtab_d = nc.dram_tensor("gtab", [VOCAB, G3], F16, kind="ExternalInput")
    whh_d = nc.dram_tensor("whh", [128, H2 * GB * 128], F16, kind="ExternalInput")
    bhhn_d = nc.dram_tensor("bhhn", [128, H2], F32, kind="ExternalInput")
    xw_d = nc.dram_tensor("xw", [128, S * BCORE // 16], I16, kind="ExternalInput")
    wfc_d = nc.dram_tensor("wfc", [128, H2 * VOCAB], F16, kind="ExternalInput")
    bfc_d = nc.dram_tensor("bfc", [1, VOCAB], F16, kind="ExternalInput")
    logits_d = nc.dram_tensor("logits", [S, BCORE, VOCAB], F32, kind="ExternalOutput")
    hlast_d = nc.dram_tensor("hlast", [128, NG * BCORE], F32, kind="ExternalOutput")

    with tile.TileContext(nc) as tc:
        _emit_recurrence(
            nc, tc, S, gtab_d, xw_d, whh_d, bhhn_d, wfc_d, bfc_d, logits_d, hlast_d
        )

    nc.compile()
    return nc


def _prep_shared(embed, W_ih, b_ih, W_hh, b_hh, W_fc, b_fc):
    """Host-side weight repacking (input-independent)."""
    G = embed.astype(np.float64) @ W_ih.T.astype(np.float64) + b_ih
    G[:, : 2 * HIDDEN] += b_hh[: 2 * HIDDEN]
    gtab = G.astype(np.float16)

    # whh[p, (eta*GB+gb)*128 + m] = W_hh[gb*128+m, eta*128+p]
    w4 = W_hh.reshape(GB, 128, H2, 128)  # [gb, m, eta, p]
    whh = np.ascontiguousarray(
        w4.transpose(3, 2, 0, 1).reshape(128, H2 * GB * 128)
    ).astype(np.float16)

    bhhn = np.ascontiguousarray(
        b_hh[2 * HIDDEN :].reshape(H2, 128).T
    ).astype(np.float32)

    # wfc[p, eta*V + v] = W_fc[v, eta*128+p]
    wf = W_fc.T.reshape(H2, 128, VOCAB)  # [eta, p, v]
    wfc = np.ascontiguousarray(wf.transpose(1, 0, 2).reshape(128, H2 * VOCAB)).astype(
        np.float16
    )

    bfc = b_fc.astype(np.float16).reshape(1, VOCAB)
    return gtab, whh, bhhn, wfc, bfc


def _prep_indices(xc, S):
    """Wrapped int16 gather indices for one core's batch slice [BCORE, S]."""
    nch = S // CHUNK
    # flat[call, i] with call=(g, chunk), i = tl*BG + b -> x[g*BG+b, c*CHUNK+tl]
    xr = xc.reshape(NG, BG, nch, CHUNK)  # [g, b, c, tl]
    flat = xr.transpose(0, 2, 3, 1).reshape(NG * nch, CHUNK * BG)
    # per call: wrapped[p, s] = flat[s*16 + p]
    wrapped = flat.reshape(NG * nch, CHUNK * BG // 16, 16).transpose(0, 2, 1)
    xw = wrapped.transpose(1, 0, 2).reshape(16, NG * nch * (CHUNK * BG // 16))
    return np.ascontiguousarray(np.tile(xw, (8, 1))).astype(np.int16)


def kernel(x, embed, W_ih, b_ih, W_hh, b_hh, W_fc, b_fc):
    x = np.asarray(x)
    embed = np.asarray(embed, dtype=np.float32)
    W_ih = np.asarray(W_ih, dtype=np.float32)
    b_ih = np.asarray(b_ih, dtype=np.float32)
    W_hh = np.asarray(W_hh, dtype=np.float32)
    b_hh = np.asarray(b_hh, dtype=np.float32)
    W_fc = np.asarray(W_fc, dtype=np.float32)
    b_fc = np.asarray(b_fc, dtype=np.float32)

    B, S = x.shape
    assert B == BATCH and S == SEQ, (B, S)

    gtab, whh, bhhn, wfc, bfc = _prep_shared(
        embed, W_ih, b_ih, W_hh, b_hh, W_fc, b_fc
    )

    nc = _build(S)

    in_maps = []
    for c in range(NCORES):
        xc = x[c * BCORE : (c + 1) * BCORE].astype(np.int64)
        in_maps.append(
            {
                "gtab": gtab,
                "whh": whh,
                "bhhn": bhhn,
                "xw": _prep_indices(xc, S),
                "wfc": wfc,
                "bfc": bfc,
            }
        )

    from concourse.bass_utils import run_bass_kernel_spmd

    res = run_bass_kernel_spmd(nc, in_maps, core_ids=list(range(NCORES)))
    global LAST_RESULTS
    LAST_RESULTS = res

    logits = np.empty((BATCH, S, VOCAB), dtype=np.float32)
    h_last = np.empty((BATCH, HIDDEN), dtype=np.float32)
    for c in range(NCORES):
        lg = res.results[c]["logits"]  # [S, BCORE, V]
        hl = res.results[c]["hlast"]  # [128, NG*BCORE] cols g*8 + eta*4 + b
        logits[c * BCORE : (c + 1) * BCORE] = lg.transpose(1, 0, 2)
        for g in range(NG):
            for eta in range(H2):
                for b in range(BG):
                    h_last[
                        c * BCORE + g * BG + b, eta * 128 : (eta + 1) * 128
                    ] = hl[:, g * BCORE + eta * BG + b]
    return logits, h_last
